# revision 1
# baseline (speedup 1.0000x reference)
"""RoIAlign-style crop+bilinear-resize kernel for Trainium2 (8 NeuronCores).

Strategy (per core, 64 boxes):
  - Host precomputes, per box: gather indices (row*16 + xblock, int16),
    a 2-sparse horizontal resample matrix Wx (bf16), and vertical lerp
    weights wy (f32).
  - dma_gather(transpose=True) pulls, for each of the 512 needed source
    rows (Y0 pairs then Y1 pairs), an nx*128-wide bf16 column span; the
    transpose puts image-x on SBUF partitions.
  - PE matmul contracts x: psA = A0.T @ Wx, psB = A1.T @ Wx (K=128 chunks
    accumulated in PSUM), putting output rows s on partitions.
  - DVE does the vertical lerp on PSUM: out = psA + wy[s]*(psB-psA).
  - One program is shared SPMD across 8 cores: all per-box geometry lives
    in input data; boxes are sorted by width and dealt round-robin so the
    per-slot padded width nx matches across cores.
"""

import os
import numpy as np
import ml_dtypes

C, H, W, N, S = 3, 2048, 2048, 512, 256
NCORES = 8
NB = N // NCORES          # boxes per core
BLK = 128                 # x-block granularity (elements)
NBLK = W // BLK           # 16
IMGLEN = H * W + W        # one pad row
NIDX = 2 * S              # gather indices per channel (Y0 rows + Y1 rows)
BF16 = ml_dtypes.bfloat16

_NC_CACHE = {}


def _axis_coords(lo, size, s):
    """float32 mirror of reference._axis_coords."""
    scale = size.astype(np.float32) / np.float32(s)
    src = (np.arange(s, dtype=np.float32)[None, :] + np.float32(0.5)) * scale[:, None] \
        - np.float32(0.5)
    max_i = np.maximum(size[:, None] - 1, 0)
    src = np.clip(src, np.float32(0.0), max_i.astype(np.float32))
    i0 = np.floor(src).astype(np.int32)
    i1 = np.minimum(i0 + 1, max_i)
    w = src - i0.astype(np.float32)
    return lo[:, None] + i0, lo[:, None] + i1, w


def _prep(image, boxes):
    """Host-side preprocessing. Returns per-core input maps + assignment."""
    boxes = boxes.astype(np.int32)
    x1 = np.clip(boxes[:, 0], 0, W); y1 = np.clip(boxes[:, 1], 0, H)
    x2 = np.clip(boxes[:, 2], 0, W); y2 = np.clip(boxes[:, 3], 0, H)
    bw = x2 - x1; bh = y2 - y1
    valid = (bw > 0) & (bh > 0)
    X0, X1, wx = _axis_coords(x1, bw, S)
    Y0, Y1, wy = _axis_coords(y1, bh, S)
    X0 = np.clip(X0, 0, W - 1); X1 = np.clip(X1, 0, W - 1)
    Y0 = np.clip(Y0, 0, H - 1); Y1 = np.clip(Y1, 0, H - 1)

    xb0n = np.where(valid, np.minimum(x1, W - 1) // BLK, 0)
    nxn = np.where(valid, (x2 - xb0n * BLK + BLK - 1) // BLK, 1).astype(np.int64)
    nxn = np.maximum(nxn, 1)

    # widest boxes first; deal round-robin so slot j is similar across cores
    order = np.argsort(-nxn, kind="stable")
    asg = order.reshape(NB, NCORES)          # [slot, core] -> box id
    nx_slot = nxn[asg].max(axis=1)           # [slot]
    wx_off = np.concatenate([[0], np.cumsum(nx_slot)])  # block offsets per slot

    # bf16 images (shared by all cores)
    imgs = []
    for c in range(C):
        buf = np.zeros(IMGLEN, dtype=BF16)
        buf[: H * W] = image[c].astype(BF16).ravel()
        imgs.append(buf)

    in_maps = []
    for k in range(NCORES):
        idxb = np.zeros((NB, 128, NIDX // 16), dtype=np.int16)
        wyb = np.zeros((NB, 128, 4), dtype=np.float32)
        wxb = np.zeros((int(wx_off[-1]) * 128 * S,), dtype=BF16)
        for j in range(NB):
            b = int(asg[j, k])
            nx = int(nx_slot[j])
            if valid[b]:
                xb0 = min(int(xb0n[b]), NBLK - nx)
                # gather index order i = pair*256 + s
                iv = np.empty(NIDX, dtype=np.int16)
                iv[:S] = (Y0[b] * NBLK + xb0).astype(np.int16)
                iv[S:] = (Y1[b] * NBLK + xb0).astype(np.int16)
                # wrap: idx i lives at [partition i%16, free i//16]; replicate x8
                idxb[j] = np.tile(iv.reshape(NIDX // 16, 16).T, (8, 1))
                wyb[j, :, 0:2] = wy[b].reshape(2, 128).T
                wyb[j, :, 2:4] = 1.0 - wyb[j, :, 0:2]
                # 2-sparse horizontal matrix [nx*128, 256]; per column t the
                # two target rows are distinct ops -> scatter via fancy index
                wm = np.zeros((nx * BLK, S), dtype=np.float32)
                t = np.arange(S)
                wm[X0[b] - xb0 * BLK, t] = 1.0 - wx[b]
                wm[X1[b] - xb0 * BLK, t] += wx[b]
                o = int(wx_off[j]) * 128 * S
                wxb[o: o + nx * 128 * S] = \
                    wm.reshape(nx, BLK, S).astype(BF16).ravel()
        m = {"img0": imgs[0], "img1": imgs[1], "img2": imgs[2],
             "wxb": wxb, "idxb": idxb, "wyb": wyb}
        in_maps.append(m)
    return in_maps, asg, valid, nx_slot, wx_off


def _build(nx_slot, wx_off, nb=NB, repeat=1):
    import contextlib
    import concourse.bacc as bacc
    import concourse.mybir as mybir
    from concourse.tile import TileContext

    dt = mybir.dt
    nc = bacc.Bacc("TRN2", target_bir_lowering=False, debug=False,
                   enable_asserts=False, num_devices=NCORES)
    imgs = [nc.dram_tensor(f"img{c}", [IMGLEN], dt.bfloat16,
                           kind="ExternalInput").ap() for c in range(C)]
    wxtot = int(wx_off[-1]) * 128 * S
    wxb = nc.dram_tensor("wxb", [wxtot], dt.bfloat16,
                         kind="ExternalInput").ap()
    idxb = nc.dram_tensor("idxb", [NB, 128, NIDX // 16], dt.int16,
                          kind="ExternalInput").ap()
    wyb = nc.dram_tensor("wyb", [NB, 128, 4], dt.float32,
                         kind="ExternalInput").ap()
    outp = nc.dram_tensor("out", [NB, C, 2, 128, S], dt.float32,
                          kind="ExternalOutput").ap()

    # gather source view: rows of 128 elements, element spans nx*128 (overlap)
    def img_src_ap(c, nx):
        a = imgs[c].rearrange("(r e) -> r e", e=BLK)
        a = a.copy()
        ap = a.ap
        ap[0] = [BLK, H * W // BLK]
        ap[-1] = [1, nx * BLK]
        a.ap = ap
        return a

    with TileContext(nc) as tc:
        with tc.tile_pool(name="io", bufs=3) as iop, \
             tc.tile_pool(name="tp", bufs=2) as tpool, \
             tc.tile_pool(name="ps", bufs=2, space="PSUM") as psp, \
             (tc.For_i(0, repeat, 1) if repeat > 1
              else contextlib.nullcontext()):
            for j in range(nb):
                nx = int(nx_slot[j])
                idxt = iop.tile([128, NIDX // 16], dt.int16, tag="idx")
                nc.sync.dma_start(out=idxt[:], in_=idxb[j])
                wyt = iop.tile([128, 4], dt.float32, tag="wy")
                nc.sync.dma_start(out=wyt[:], in_=wyb[j])
                wxt = tpool.tile([128, nx, S], dt.bfloat16, tag="wx")
                o = int(wx_off[j]) * 128 * S
                nc.sync.dma_start(
                    out=wxt[:],
                    in_=wxb[o: o + nx * 128 * S].rearrange(
                        "(cx p t) -> p cx t", p=128, t=S))
                timg = tpool.tile([128, C, nx, NIDX], dt.bfloat16, tag="timg")
                for c in range(C):
                    nc.gpsimd.dma_gather(
                        out_ap=timg[:, c, :, :],
                        in_ap=img_src_ap(c, nx),
                        idxs_ap=idxt[:],
                        num_idxs=NIDX,
                        num_idxs_reg=NIDX,
                        elem_size=nx * BLK,
                        elem_step=BLK,
                        transpose=True,
                    )
                stage = tpool.tile([128, C, 2, S], dt.float32, tag="stage")
                for c in range(C):
                    psA = psp.tile([128, 2, S], dt.float32, tag="psA")
                    psB = psp.tile([128, 2, S], dt.float32, tag="psB")
                    for sh in range(2):
                        for cx in range(nx):
                            nc.tensor.matmul(
                                psA[:, sh, :],
                                timg[:, c, cx, sh * 128:(sh + 1) * 128],
                                wxt[:, cx, :],
                                start=(cx == 0), stop=(cx == nx - 1))
                        for cx in range(nx):
                            nc.tensor.matmul(
                                psB[:, sh, :],
                                timg[:, c, cx, S + sh * 128:S + (sh + 1) * 128],
                                wxt[:, cx, :],
                                start=(cx == 0), stop=(cx == nx - 1))
                    for sh in range(2):
                        d = iop.tile([128, S], dt.float32, tag="d")
                        nc.vector.tensor_scalar_mul(
                            d[:], psB[:, sh, :], wyt[:, sh:sh + 1])
                        nc.vector.scalar_tensor_tensor(
                            stage[:, c, sh, :], psA[:, sh, :],
                            wyt[:, 2 + sh:3 + sh], d[:],
                            mybir.AluOpType.mult, mybir.AluOpType.add)
                nc.sync.dma_start(
                    out=outp[j].rearrange("c h p t -> p c h t"),
                    in_=stage[:])
    nc.compile()
    return nc


def kernel(image, boxes, crop_size):
    from concourse.bass_utils import run_bass_kernel_spmd

    image = np.asarray(image, dtype=np.float32)
    boxes = np.asarray(boxes)
    in_maps, asg, valid, nx_slot, wx_off = _prep(image, boxes)

    key = tuple(int(x) for x in nx_slot)
    if key not in _NC_CACHE:
        _NC_CACHE.clear()
        _NC_CACHE[key] = _build(nx_slot, wx_off)
    nc = _NC_CACHE[key]

    trace = os.environ.get("BASS_PROFILE", "") == "1"
    res = run_bass_kernel_spmd(nc, in_maps, list(range(NCORES)), trace=trace)
    global LAST_RESULTS
    LAST_RESULTS = res

    out = np.zeros((N, C, S, S), dtype=np.float32)
    for k in range(NCORES):
        out[asg[:, k]] = res.results[k]["out"].reshape(NB, C, S, S)
    out[~valid] = 0.0
    return out


LAST_RESULTS = None

